# revision 2
# baseline (speedup 1.0000x reference)
"""Trainium2 Bass kernel for nn_ColdPrompt (dense_mlp).

Computes, for weight [B=256, P=4, D=768], W_spec [U=32, IN=3072, D=768],
b_spec [U=32, D=768]:
    prompt_emb    = weight.reshape(B, IN)                      # [256, 3072]
    task_specific = einsum('bi,uio->ubo', prompt_emb, W_spec) + b_spec[:,None,:]
                    -> reshape [U*B, D]                        # [8192, 768]
    mean_emb      = weight.mean(axis=1)                        # [256, 768]

Sharding: U sharded 4-users-per-core across 8 NeuronCores (expert-parallel),
prompt_emb replicated.  Per core the GEMM is [256,3072] @ [3072,768] x4 users,
computed on the PE with prompt_emb^T as the stationary operand (reused across
users / output columns) and W streamed from HBM exactly once.

Precision modes:
  "bf16"   - both operands rounded to bf16, fp32 PSUM accumulate (~2.4e-3 rel)
  "bf16x3" - hi/lo bf16 split of both operands, 3 accumulating passes
             (x_hi*W_hi + x_hi*W_lo + x_lo*W_hi), ~5e-6 rel, 3x PE work
"""

import os

import numpy as np
import ml_dtypes

import concourse.bass as bass
import concourse.bacc as bacc
import concourse.mybir as mybir
import concourse.tile as tile
from concourse.bass_utils import run_bass_kernel_spmd

BF16 = ml_dtypes.bfloat16

# Problem shapes (hardcoded per the contract).
B, P, D = 256, 4, 768
U = 32
IN = P * D            # 3072
NCORES = 8
UPC = U // NCORES     # users per core = 4
KT = IN // 128        # 24 contraction k-tiles
KB = 4                # k-tiles per W DMA block
NB = KT // KB         # 6 blocks
MT = B // 128         # 2 output row tiles
NW = 384              # matmul moving free dim (psum bank: <=512 fp32)
NT = D // NW          # 2 output col tiles

MODE = os.environ.get("CC_KERNEL_MODE", "bf16x3")

_LAST_PERF = {}
_NC_CACHE = {}


def _build_nc(mode: str):
    """Build + compile the (SPMD, per-core) Bass module."""
    nxt = 2 if mode == "bf16x3" else 1   # number of distinct x operands
    nwt = 2 if mode == "bf16x3" else 1   # number of distinct W streams
    # (x_idx, w_idx) per accumulation pass, ordered so same-x passes are
    # adjacent (stationary-operand reuse on the PE).
    passes = [(0, 0), (0, 1), (1, 0)] if mode == "bf16x3" else [(0, 0)]

    nc = bacc.Bacc(
        "TRN2",
        target_bir_lowering=False,
        debug=False,
        enable_asserts=False,
        num_devices=NCORES,
    )

    f32 = mybir.dt.float32
    bf16 = mybir.dt.bfloat16

    xt_d = [
        nc.dram_tensor(f"xt{i}", [128, KT, B], bf16, kind="ExternalInput").ap()
        for i in range(nxt)
    ]
    w_d = [
        nc.dram_tensor(f"w{i}", [UPC, NB, 128, KB * D], bf16, kind="ExternalInput").ap()
        for i in range(nwt)
    ]
    bias_d = nc.dram_tensor("bias", [128, UPC, D], f32, kind="ExternalInput").ap()
    wf_d = nc.dram_tensor("wf", [MT, 128, IN], f32, kind="ExternalInput").ap()

    task_d = nc.dram_tensor(
        "task_out", [UPC, MT, 128, D], f32, kind="ExternalOutput"
    ).ap()
    mean_d = nc.dram_tensor("mean_out", [MT, 128, D], f32, kind="ExternalOutput").ap()

    with tile.TileContext(nc) as tc:
        with (
            tc.tile_pool(name="const", bufs=1) as cpool,
            tc.tile_pool(name="wstream", bufs=3) as wpool,
            tc.tile_pool(name="out", bufs=4) as opool,
            tc.tile_pool(name="meanbuf", bufs=2) as mpool,
            tc.tile_pool(name="psum", bufs=8, space=bass.MemorySpace.PSUM) as ppool,
        ):
            # Resident constants: x^T tiles and the (host-pre-broadcast) bias.
            xt_sb = []
            for i in range(nxt):
                t = cpool.tile([128, KT, B], bf16, tag=f"xt{i}", name=f"xt{i}_sb")
                nc.sync.dma_start(t[:], xt_d[i][:])
                xt_sb.append(t)
            bias_sb = cpool.tile([128, UPC, D], f32, tag="bias")
            nc.sync.dma_start(bias_sb[:], bias_d[:])

            # Main per-user GEMM: accumulate over all k into 4 psum tiles
            # (2 row-tiles x 2 col-tiles), W streamed HBM->SBUF per k-block.
            for u in range(UPC):
                ps = [
                    [ppool.tile([128, NW], f32, tag="ps", name="ps") for _ in range(NT)]
                    for _ in range(MT)
                ]
                for kb in range(NB):
                    w_sb = []
                    for wi in range(nwt):
                        t = wpool.tile([128, KB, D], bf16, tag=f"w{wi}", name=f"w{wi}_sb")
                        nc.sync.dma_start(t[:], w_d[wi][u, kb])
                        w_sb.append(t)
                    for ki in range(KB):
                        k = kb * KB + ki
                        for m in range(MT):
                            for pi, (xi, wi) in enumerate(passes):
                                lhsT = xt_sb[xi][:, k, m * 128 : (m + 1) * 128]
                                for n in range(NT):
                                    nc.tensor.matmul(
                                        ps[m][n][:],
                                        lhsT,
                                        w_sb[wi][:, ki, n * NW : (n + 1) * NW],
                                        start=(k == 0 and pi == 0),
                                        stop=(k == KT - 1 and pi == len(passes) - 1),
                                    )
                # Drain psum -> sbuf with bias add, then DMA out.
                for m in range(MT):
                    ot = opool.tile([128, D], f32, tag="ot")
                    for n in range(NT):
                        sl = slice(n * NW, (n + 1) * NW)
                        nc.vector.tensor_add(ot[:, sl], ps[m][n][:], bias_sb[:, u, sl])
                    nc.sync.dma_start(task_d[u, m], ot[:])

            # mean_emb = weight.mean(axis=1), fp32 end to end.
            for m in range(MT):
                wf_sb = mpool.tile([128, IN], f32, tag="wf")
                nc.sync.dma_start(wf_sb[:], wf_d[m])
                t0 = mpool.tile([128, D], f32, tag="mt0")
                t1 = mpool.tile([128, D], f32, tag="mt1")
                mo = mpool.tile([128, D], f32, tag="mo")
                nc.vector.tensor_add(t0[:], wf_sb[:, 0:D], wf_sb[:, D : 2 * D])
                nc.vector.tensor_add(t1[:], wf_sb[:, 2 * D : 3 * D], wf_sb[:, 3 * D :])
                nc.vector.tensor_add(t0[:], t0[:], t1[:])
                nc.scalar.mul(mo[:], t0[:], 0.25)
                nc.sync.dma_start(mean_d[m], mo[:])

    nc.compile()
    return nc


def _split_hi_lo(a):
    hi = a.astype(BF16)
    lo = (a - hi.astype(np.float32)).astype(BF16)
    return hi, lo


def kernel(weight, W_spec, b_spec):
    mode = MODE
    if mode not in _NC_CACHE:
        _NC_CACHE[mode] = _build_nc(mode)
    nc = _NC_CACHE[mode]

    weight = np.asarray(weight, dtype=np.float32)
    W_spec = np.asarray(W_spec, dtype=np.float32)
    b_spec = np.asarray(b_spec, dtype=np.float32)

    x = weight.reshape(B, IN)
    # xt[p, k, b] = x[b, 128k + p]
    xt_f32 = np.ascontiguousarray(x.T).reshape(KT, 128, B).transpose(1, 0, 2)
    if mode == "bf16x3":
        xh, xl = _split_hi_lo(xt_f32)
        xts = [np.ascontiguousarray(xh), np.ascontiguousarray(xl)]
        Wh, Wl = _split_hi_lo(W_spec)
        Ws = [Wh, Wl]
    else:
        xts = [np.ascontiguousarray(xt_f32.astype(BF16))]
        Ws = [W_spec.astype(BF16)]

    wf = np.ascontiguousarray(weight.reshape(MT, 128, IN))

    in_maps = []
    for c in range(NCORES):
        us = slice(c * UPC, (c + 1) * UPC)
        m = {}
        for i, xti in enumerate(xts):
            m[f"xt{i}"] = xti
        for i, Wi in enumerate(Ws):
            # w[u, kb, p, ki*D + d] = W[c*UPC+u, (KB*kb+ki)*128 + p, d]
            wc = (
                Wi[us]
                .reshape(UPC, NB, KB, 128, D)
                .transpose(0, 1, 3, 2, 4)
                .reshape(UPC, NB, 128, KB * D)
            )
            m[f"w{i}"] = np.ascontiguousarray(wc)
        m["bias"] = np.ascontiguousarray(
            np.broadcast_to(b_spec[us][None, :, :], (128, UPC, D))
        )
        m["wf"] = wf
        in_maps.append(m)

    res = run_bass_kernel_spmd(
        nc,
        in_maps,
        core_ids=list(range(NCORES)),
        trace=bool(int(os.environ.get("CC_KERNEL_TRACE", "0"))),
    )
    _LAST_PERF.clear()
    _LAST_PERF.update(
        exec_time_ns=res.exec_time_ns,
        mean_exec_time_ns=res.mean_exec_time_ns,
        max_exec_time_core_id=res.max_exec_time_core_id,
        trace=res.instructions_and_trace[1] if res.instructions_and_trace else None,
    )

    task = np.concatenate(
        [res.results[c]["task_out"].reshape(UPC * B, D) for c in range(NCORES)], axis=0
    )
    mean = res.results[0]["mean_out"].reshape(B, D)
    return task.astype(np.float32, copy=False), mean.astype(np.float32, copy=False)


# revision 5
# speedup vs baseline: 60854.8460x; 60854.8460x over previous
"""Trainium2 Bass kernel for nn_ColdPrompt (dense_mlp).

Computes, for weight [B=256, P=4, D=768], W_spec [U=32, IN=3072, D=768],
b_spec [U=32, D=768]:
    prompt_emb    = weight.reshape(B, IN)                      # [256, 3072]
    task_specific = einsum('bi,uio->ubo', prompt_emb, W_spec) + b_spec[:,None,:]
                    -> reshape [U*B, D]                        # [8192, 768]
    mean_emb      = weight.mean(axis=1)                        # [256, 768]

Sharding: U sharded 4-users-per-core across 8 NeuronCores (expert-parallel),
prompt_emb replicated.  Per core the GEMM is [256,3072] @ [3072,768] x4 users,
computed on the PE with prompt_emb^T as the stationary operand (reused across
users / output columns) and W streamed from HBM exactly once.

Precision modes:
  "bf16"   - both operands rounded to bf16, fp32 PSUM accumulate (~2.4e-3 rel)
  "bf16x3" - hi/lo bf16 split of both operands, 3 accumulating passes
             (x_hi*W_hi + x_hi*W_lo + x_lo*W_hi), ~5e-6 rel, 3x PE work
"""

import os

import numpy as np
import ml_dtypes

import concourse.bass as bass
import concourse.bacc as bacc
import concourse.mybir as mybir
import concourse.tile as tile
from concourse.bass_utils import run_bass_kernel_spmd

BF16 = ml_dtypes.bfloat16

# Problem shapes (hardcoded per the contract).
B, P, D = 256, 4, 768
U = 32
IN = P * D            # 3072
NCORES = 8
UPC = U // NCORES     # users per core = 4
KT = IN // 128        # 24 contraction k-tiles
KB = 4                # k-tiles per W DMA block
NB = KT // KB         # 6 blocks
MT = B // 128         # 2 output row tiles
NW = 384              # matmul moving free dim (psum bank: <=512 fp32)
NT = D // NW          # 2 output col tiles

MODE = os.environ.get("CC_KERNEL_MODE", "bf16x3")

_LAST_PERF = {}
_NC_CACHE = {}


def _build_nc(mode: str, reps: int = 1):
    """Build + compile the (SPMD, per-core) Bass module.

    reps>1 repeats the whole compute body (same inputs/outputs) so marginal
    per-rep time can be measured free of launch/transfer overhead."""
    nxt = 2 if mode == "bf16x3" else 1   # number of distinct x operands
    nwt = 2 if mode == "bf16x3" else 1   # number of distinct W streams
    # (x_idx, w_idx) per accumulation pass, ordered so same-x passes are
    # adjacent (stationary-operand reuse on the PE).
    passes = [(0, 0), (0, 1), (1, 0)] if mode == "bf16x3" else [(0, 0)]

    nc = bacc.Bacc(
        "TRN2",
        target_bir_lowering=False,
        debug=False,
        enable_asserts=False,
        num_devices=NCORES,
    )

    f32 = mybir.dt.float32
    bf16 = mybir.dt.bfloat16

    xt_d = [
        nc.dram_tensor(f"xt{i}", [128, KT, B], bf16, kind="ExternalInput").ap()
        for i in range(nxt)
    ]
    w_d = [
        nc.dram_tensor(f"w{i}", [UPC, NB, 128, KB * D], bf16, kind="ExternalInput").ap()
        for i in range(nwt)
    ]
    bias_d = nc.dram_tensor("bias", [128, UPC, D], f32, kind="ExternalInput").ap()
    wf_d = nc.dram_tensor("wf", [MT, 128, IN], f32, kind="ExternalInput").ap()

    task_d = nc.dram_tensor(
        "task_out", [UPC, MT, 128, D], f32, kind="ExternalOutput"
    ).ap()
    mean_d = nc.dram_tensor("mean_out", [MT, 128, D], f32, kind="ExternalOutput").ap()

    with tile.TileContext(nc) as tc:
        with (
            tc.tile_pool(name="const", bufs=1) as cpool,
            tc.tile_pool(name="wstream", bufs=3) as wpool,
            tc.tile_pool(name="out", bufs=4) as opool,
            tc.tile_pool(name="meanbuf", bufs=2) as mpool,
            tc.tile_pool(name="psum", bufs=8, space=bass.MemorySpace.PSUM) as ppool,
        ):
            # Resident constants: x^T tiles and the (host-pre-broadcast) bias.
            xt_sb = []
            for i in range(nxt):
                t = cpool.tile([128, KT, B], bf16, tag=f"xt{i}", name=f"xt{i}_sb")
                nc.sync.dma_start(t[:], xt_d[i][:])
                xt_sb.append(t)
            bias_sb = cpool.tile([128, UPC, D], f32, tag="bias")
            nc.sync.dma_start(bias_sb[:], bias_d[:])

            # Main per-user GEMM: accumulate over all k into 4 psum tiles
            # (2 row-tiles x 2 col-tiles), W streamed HBM->SBUF per k-block.
            for _rep in range(reps):
                for u in range(UPC):
                    ps = [
                        [
                            ppool.tile([128, NW], f32, tag="ps", name="ps")
                            for _ in range(NT)
                        ]
                        for _ in range(MT)
                    ]
                    for kb in range(NB):
                        w_sb = []
                        for wi in range(nwt):
                            t = wpool.tile(
                                [128, KB, D], bf16, tag=f"w{wi}", name=f"w{wi}_sb"
                            )
                            nc.sync.dma_start(t[:], w_d[wi][u, kb])
                            w_sb.append(t)
                        for ki in range(KB):
                            k = kb * KB + ki
                            for m in range(MT):
                                for pi, (xi, wi) in enumerate(passes):
                                    lhsT = xt_sb[xi][:, k, m * 128 : (m + 1) * 128]
                                    for n in range(NT):
                                        nc.tensor.matmul(
                                            ps[m][n][:],
                                            lhsT,
                                            w_sb[wi][:, ki, n * NW : (n + 1) * NW],
                                            start=(k == 0 and pi == 0),
                                            stop=(
                                                k == KT - 1 and pi == len(passes) - 1
                                            ),
                                        )
                    # Drain psum -> sbuf with bias add, then DMA out.
                    for m in range(MT):
                        ot = opool.tile([128, D], f32, tag="ot")
                        for n in range(NT):
                            sl = slice(n * NW, (n + 1) * NW)
                            nc.vector.tensor_add(
                                ot[:, sl], ps[m][n][:], bias_sb[:, u, sl]
                            )
                        nc.sync.dma_start(task_d[u, m], ot[:])

                # mean_emb = weight.mean(axis=1), fp32 end to end.
                for m in range(MT):
                    wf_sb = mpool.tile([128, IN], f32, tag="wf")
                    nc.sync.dma_start(wf_sb[:], wf_d[m])
                    t0 = mpool.tile([128, D], f32, tag="mt0")
                    t1 = mpool.tile([128, D], f32, tag="mt1")
                    mo = mpool.tile([128, D], f32, tag="mo")
                    nc.vector.tensor_add(t0[:], wf_sb[:, 0:D], wf_sb[:, D : 2 * D])
                    nc.vector.tensor_add(
                        t1[:], wf_sb[:, 2 * D : 3 * D], wf_sb[:, 3 * D :]
                    )
                    nc.vector.tensor_add(t0[:], t0[:], t1[:])
                    nc.scalar.mul(mo[:], t0[:], 0.25)
                    nc.sync.dma_start(mean_d[m], mo[:])

    nc.compile()
    return nc


def _split_hi_lo(a):
    hi = a.astype(BF16)
    lo = (a - hi.astype(np.float32)).astype(BF16)
    return hi, lo


def kernel(weight, W_spec, b_spec):
    mode = MODE
    if mode not in _NC_CACHE:
        _NC_CACHE[mode] = _build_nc(mode)
    nc = _NC_CACHE[mode]

    weight = np.asarray(weight, dtype=np.float32)
    W_spec = np.asarray(W_spec, dtype=np.float32)
    b_spec = np.asarray(b_spec, dtype=np.float32)

    x = weight.reshape(B, IN)
    # xt[p, k, b] = x[b, 128k + p]
    xt_f32 = np.ascontiguousarray(x.T).reshape(KT, 128, B).transpose(1, 0, 2)
    if mode == "bf16x3":
        xh, xl = _split_hi_lo(xt_f32)
        xts = [np.ascontiguousarray(xh), np.ascontiguousarray(xl)]
        Wh, Wl = _split_hi_lo(W_spec)
        Ws = [Wh, Wl]
    else:
        xts = [np.ascontiguousarray(xt_f32.astype(BF16))]
        Ws = [W_spec.astype(BF16)]

    wf = np.ascontiguousarray(weight.reshape(MT, 128, IN))

    in_maps = []
    for c in range(NCORES):
        us = slice(c * UPC, (c + 1) * UPC)
        m = {}
        for i, xti in enumerate(xts):
            m[f"xt{i}"] = xti
        for i, Wi in enumerate(Ws):
            # w[u, kb, p, ki*D + d] = W[c*UPC+u, (KB*kb+ki)*128 + p, d]
            wc = (
                Wi[us]
                .reshape(UPC, NB, KB, 128, D)
                .transpose(0, 1, 3, 2, 4)
                .reshape(UPC, NB, 128, KB * D)
            )
            m[f"w{i}"] = np.ascontiguousarray(wc)
        m["bias"] = np.ascontiguousarray(
            np.broadcast_to(b_spec[us][None, :, :], (128, UPC, D))
        )
        m["wf"] = wf
        in_maps.append(m)

    res = run_bass_kernel_spmd(
        nc,
        in_maps,
        core_ids=list(range(NCORES)),
        trace=bool(int(os.environ.get("CC_KERNEL_TRACE", "0"))),
    )
    _LAST_PERF.clear()
    _LAST_PERF.update(
        exec_time_ns=res.exec_time_ns,
        mean_exec_time_ns=res.mean_exec_time_ns,
        max_exec_time_core_id=res.max_exec_time_core_id,
        trace=res.instructions_and_trace[1] if res.instructions_and_trace else None,
    )

    task = np.concatenate(
        [res.results[c]["task_out"].reshape(UPC * B, D) for c in range(NCORES)], axis=0
    )
    mean = res.results[0]["mean_out"].reshape(B, D)
    return task.astype(np.float32, copy=False), mean.astype(np.float32, copy=False)


# revision 7
# speedup vs baseline: 136689.2500x; 2.2462x over previous
"""Trainium2 Bass kernel for nn_ColdPrompt (dense_mlp).

Computes, for weight [B=256, P=4, D=768], W_spec [U=32, IN=3072, D=768],
b_spec [U=32, D=768]:
    prompt_emb    = weight.reshape(B, IN)                      # [256, 3072]
    task_specific = einsum('bi,uio->ubo', prompt_emb, W_spec) + b_spec[:,None,:]
                    -> reshape [U*B, D]                        # [8192, 768]
    mean_emb      = weight.mean(axis=1)                        # [256, 768]

Sharding: U sharded 4-users-per-core across 8 NeuronCores (expert-parallel),
prompt_emb replicated.  Per core the GEMM is [256,3072] @ [3072,768] x4 users,
computed on the PE with prompt_emb^T as the stationary operand (reused across
users / output columns) and W streamed from HBM exactly once.

Precision modes:
  "fp16"   - both operands rounded to fp16, fp32 PSUM accumulate (~3e-4 rel),
             full PE rate (1 cycle/row), same DMA volume as bf16
  "bf16"   - both operands rounded to bf16, fp32 PSUM accumulate (~2.4e-3 rel)
  "bf16x3" - hi/lo bf16 split of both operands, 3 accumulating passes
             (x_hi*W_hi + x_hi*W_lo + x_lo*W_hi), ~5e-6 rel, 3x PE work
"""

import os

import numpy as np
import ml_dtypes

import concourse.bass as bass
import concourse.bacc as bacc
import concourse.mybir as mybir
import concourse.tile as tile
from concourse.bass_utils import run_bass_kernel_spmd

BF16 = ml_dtypes.bfloat16

# Problem shapes (hardcoded per the contract).
B, P, D = 256, 4, 768
U = 32
IN = P * D            # 3072
NCORES = 8
UPC = U // NCORES     # users per core = 4
KT = IN // 128        # 24 contraction k-tiles
KB = 4                # k-tiles per W DMA block
NB = KT // KB         # 6 blocks
MT = B // 128         # 2 output row tiles
NW = 384              # matmul moving free dim (psum bank: <=512 fp32)
NT = D // NW          # 2 output col tiles

MODE = os.environ.get("CC_KERNEL_MODE", "fp16")

_LAST_PERF = {}
_NC_CACHE = {}


def _build_nc(mode: str, reps: int = 1):
    """Build + compile the (SPMD, per-core) Bass module.

    reps>1 repeats the whole compute body (same inputs/outputs) so marginal
    per-rep time can be measured free of launch/transfer overhead."""
    nxt = 2 if mode == "bf16x3" else 1   # number of distinct x operands
    nwt = 2 if mode == "bf16x3" else 1   # number of distinct W streams
    mm_np = np.float16 if mode == "fp16" else BF16
    # (x_idx, w_idx) per accumulation pass, ordered so same-x passes are
    # adjacent (stationary-operand reuse on the PE).
    passes = [(0, 0), (0, 1), (1, 0)] if mode == "bf16x3" else [(0, 0)]

    nc = bacc.Bacc(
        "TRN2",
        target_bir_lowering=False,
        debug=False,
        enable_asserts=False,
        num_devices=NCORES,
    )

    f32 = mybir.dt.float32
    bf16 = mybir.dt.float16 if mode == "fp16" else mybir.dt.bfloat16

    xt_d = [
        nc.dram_tensor(f"xt{i}", [128, KT, B], bf16, kind="ExternalInput").ap()
        for i in range(nxt)
    ]
    w_d = [
        nc.dram_tensor(f"w{i}", [UPC, NB, 128, KB * D], bf16, kind="ExternalInput").ap()
        for i in range(nwt)
    ]
    bias_d = nc.dram_tensor("bias", [128, UPC, D], f32, kind="ExternalInput").ap()
    wf_d = nc.dram_tensor("wf", [MT, 128, IN], f32, kind="ExternalInput").ap()

    task_d = nc.dram_tensor(
        "task_out", [UPC, MT, 128, D], f32, kind="ExternalOutput"
    ).ap()
    mean_d = nc.dram_tensor("mean_out", [MT, 128, D], f32, kind="ExternalOutput").ap()

    with tile.TileContext(nc) as tc:
        with (
            tc.tile_pool(name="const", bufs=1) as cpool,
            tc.tile_pool(name="wstream", bufs=3) as wpool,
            tc.tile_pool(name="out", bufs=4) as opool,
            tc.tile_pool(name="meanbuf", bufs=2) as mpool,
            tc.tile_pool(name="psum", bufs=8, space=bass.MemorySpace.PSUM) as ppool,
        ):
            # Resident constants: x^T tiles and the (host-pre-broadcast) bias.
            xt_sb = []
            for i in range(nxt):
                t = cpool.tile([128, KT, B], bf16, tag=f"xt{i}", name=f"xt{i}_sb")
                nc.sync.dma_start(t[:], xt_d[i][:])
                xt_sb.append(t)
            bias_sb = cpool.tile([128, UPC, D], f32, tag="bias")
            nc.sync.dma_start(bias_sb[:], bias_d[:])

            # Main per-user GEMM: accumulate over all k into 4 psum tiles
            # (2 row-tiles x 2 col-tiles), W streamed HBM->SBUF per k-block.
            for _rep in range(reps):
                for u in range(UPC):
                    ps = [
                        [
                            ppool.tile([128, NW], f32, tag="ps", name="ps")
                            for _ in range(NT)
                        ]
                        for _ in range(MT)
                    ]
                    for kb in range(NB):
                        w_sb = []
                        for wi in range(nwt):
                            t = wpool.tile(
                                [128, KB, D], bf16, tag=f"w{wi}", name=f"w{wi}_sb"
                            )
                            nc.sync.dma_start(t[:], w_d[wi][u, kb])
                            w_sb.append(t)
                        for ki in range(KB):
                            k = kb * KB + ki
                            for m in range(MT):
                                for pi, (xi, wi) in enumerate(passes):
                                    lhsT = xt_sb[xi][:, k, m * 128 : (m + 1) * 128]
                                    for n in range(NT):
                                        nc.tensor.matmul(
                                            ps[m][n][:],
                                            lhsT,
                                            w_sb[wi][:, ki, n * NW : (n + 1) * NW],
                                            start=(k == 0 and pi == 0),
                                            stop=(
                                                k == KT - 1 and pi == len(passes) - 1
                                            ),
                                        )
                    # Drain psum -> sbuf with bias add, then DMA out.
                    for m in range(MT):
                        ot = opool.tile([128, D], f32, tag="ot")
                        for n in range(NT):
                            sl = slice(n * NW, (n + 1) * NW)
                            nc.vector.tensor_add(
                                ot[:, sl], ps[m][n][:], bias_sb[:, u, sl]
                            )
                        nc.sync.dma_start(task_d[u, m], ot[:])

                # mean_emb = weight.mean(axis=1), fp32 end to end.
                for m in range(MT):
                    wf_sb = mpool.tile([128, IN], f32, tag="wf")
                    nc.sync.dma_start(wf_sb[:], wf_d[m])
                    t0 = mpool.tile([128, D], f32, tag="mt0")
                    t1 = mpool.tile([128, D], f32, tag="mt1")
                    mo = mpool.tile([128, D], f32, tag="mo")
                    nc.vector.tensor_add(t0[:], wf_sb[:, 0:D], wf_sb[:, D : 2 * D])
                    nc.vector.tensor_add(
                        t1[:], wf_sb[:, 2 * D : 3 * D], wf_sb[:, 3 * D :]
                    )
                    nc.vector.tensor_add(t0[:], t0[:], t1[:])
                    nc.scalar.mul(mo[:], t0[:], 0.25)
                    nc.sync.dma_start(mean_d[m], mo[:])

    nc.compile()
    return nc


def _split_hi_lo(a):
    hi = a.astype(BF16)
    lo = (a - hi.astype(np.float32)).astype(BF16)
    return hi, lo


def kernel(weight, W_spec, b_spec):
    mode = MODE
    if mode not in _NC_CACHE:
        _NC_CACHE[mode] = _build_nc(mode)
    nc = _NC_CACHE[mode]

    weight = np.asarray(weight, dtype=np.float32)
    W_spec = np.asarray(W_spec, dtype=np.float32)
    b_spec = np.asarray(b_spec, dtype=np.float32)

    x = weight.reshape(B, IN)
    # xt[p, k, b] = x[b, 128k + p]
    xt_f32 = np.ascontiguousarray(x.T).reshape(KT, 128, B).transpose(1, 0, 2)
    if mode == "bf16x3":
        xh, xl = _split_hi_lo(xt_f32)
        xts = [np.ascontiguousarray(xh), np.ascontiguousarray(xl)]
        Wh, Wl = _split_hi_lo(W_spec)
        Ws = [Wh, Wl]
    else:
        mm_np = np.float16 if mode == "fp16" else BF16
        xts = [np.ascontiguousarray(xt_f32.astype(mm_np))]
        Ws = [W_spec.astype(mm_np)]

    wf = np.ascontiguousarray(weight.reshape(MT, 128, IN))

    in_maps = []
    for c in range(NCORES):
        us = slice(c * UPC, (c + 1) * UPC)
        m = {}
        for i, xti in enumerate(xts):
            m[f"xt{i}"] = xti
        for i, Wi in enumerate(Ws):
            # w[u, kb, p, ki*D + d] = W[c*UPC+u, (KB*kb+ki)*128 + p, d]
            wc = (
                Wi[us]
                .reshape(UPC, NB, KB, 128, D)
                .transpose(0, 1, 3, 2, 4)
                .reshape(UPC, NB, 128, KB * D)
            )
            m[f"w{i}"] = np.ascontiguousarray(wc)
        m["bias"] = np.ascontiguousarray(
            np.broadcast_to(b_spec[us][None, :, :], (128, UPC, D))
        )
        m["wf"] = wf
        in_maps.append(m)

    res = run_bass_kernel_spmd(
        nc,
        in_maps,
        core_ids=list(range(NCORES)),
        trace=bool(int(os.environ.get("CC_KERNEL_TRACE", "0"))),
    )
    _LAST_PERF.clear()
    _LAST_PERF.update(
        exec_time_ns=res.exec_time_ns,
        mean_exec_time_ns=res.mean_exec_time_ns,
        max_exec_time_core_id=res.max_exec_time_core_id,
        trace=res.instructions_and_trace[1] if res.instructions_and_trace else None,
    )

    task = np.concatenate(
        [res.results[c]["task_out"].reshape(UPC * B, D) for c in range(NCORES)], axis=0
    )
    mean = res.results[0]["mean_out"].reshape(B, D)
    return task.astype(np.float32, copy=False), mean.astype(np.float32, copy=False)


# revision 9
# speedup vs baseline: 263872.7856x; 1.9305x over previous
"""Trainium2 Bass kernel for nn_ColdPrompt (dense_mlp).

Computes, for weight [B=256, P=4, D=768], W_spec [U=32, IN=3072, D=768],
b_spec [U=32, D=768]:
    prompt_emb    = weight.reshape(B, IN)                      # [256, 3072]
    task_specific = einsum('bi,uio->ubo', prompt_emb, W_spec) + b_spec[:,None,:]
                    -> reshape [U*B, D]                        # [8192, 768]
    mean_emb      = weight.mean(axis=1)                        # [256, 768]

Sharding: U sharded 4-users-per-core across 8 NeuronCores (expert-parallel),
prompt_emb replicated.  Per core the GEMM is [256,3072] @ [3072,768] x4 users,
computed on the PE with prompt_emb^T as the stationary operand (reused across
users / output columns) and W streamed from HBM exactly once.

Precision modes:
  "fp16"   - both operands rounded to fp16, fp32 PSUM accumulate (~3e-4 rel),
             full PE rate (1 cycle/row), same DMA volume as bf16
  "bf16"   - both operands rounded to bf16, fp32 PSUM accumulate (~2.4e-3 rel)
  "bf16x3" - hi/lo bf16 split of both operands, 3 accumulating passes
             (x_hi*W_hi + x_hi*W_lo + x_lo*W_hi), ~5e-6 rel, 3x PE work
"""

import os

import numpy as np
import ml_dtypes

import concourse.bass as bass
import concourse.bacc as bacc
import concourse.mybir as mybir
import concourse.tile as tile
from concourse.bass_utils import run_bass_kernel_spmd

BF16 = ml_dtypes.bfloat16

# Problem shapes (hardcoded per the contract).
B, P, D = 256, 4, 768
U = 32
IN = P * D            # 3072
NCORES = 8
UPC = U // NCORES     # users per core = 4
KT = IN // 128        # 24 contraction k-tiles
KB = 4                # k-tiles per W DMA block
NB = KT // KB         # 6 blocks
MT = B // 128         # 2 output row tiles
NW = 384              # matmul moving free dim (psum bank: <=512 fp32)
NT = D // NW          # 2 output col tiles

MODE = os.environ.get("CC_KERNEL_MODE", "fp16")

_LAST_PERF = {}
_NC_CACHE = {}


def _build_nc(mode: str, reps: int = 1):
    """Build + compile the (SPMD, per-core) Bass module.

    reps>1 repeats the whole compute body (same inputs/outputs) so marginal
    per-rep time can be measured free of launch/transfer overhead."""
    nxt = 2 if mode == "bf16x3" else 1   # number of distinct x operands
    nwt = 2 if mode == "bf16x3" else 1   # number of distinct W streams
    mm_np = np.float16 if mode == "fp16" else BF16
    # (x_idx, w_idx) per accumulation pass, ordered so same-x passes are
    # adjacent (stationary-operand reuse on the PE).
    passes = [(0, 0), (0, 1), (1, 0)] if mode == "bf16x3" else [(0, 0)]

    nc = bacc.Bacc(
        "TRN2",
        target_bir_lowering=False,
        debug=False,
        enable_asserts=False,
        num_devices=NCORES,
    )

    f32 = mybir.dt.float32
    bf16 = mybir.dt.float16 if mode == "fp16" else mybir.dt.bfloat16

    xt_d = [
        nc.dram_tensor(f"xt{i}", [128, KT, B], bf16, kind="ExternalInput").ap()
        for i in range(nxt)
    ]
    w_d = [
        nc.dram_tensor(f"w{i}", [UPC, NB, 128, KB * D], bf16, kind="ExternalInput").ap()
        for i in range(nwt)
    ]
    bias_d = nc.dram_tensor("bias", [128, UPC, D], f32, kind="ExternalInput").ap()
    # mean input: fp16 in fp16 mode (sum still fp32 on DVE, ~6e-5 rel err,
    # consistent with the task output's fp16 rounding), fp32 otherwise.
    wf_dt = bf16 if mode == "fp16" else f32
    wf_d = nc.dram_tensor("wf", [MT, 128, IN], wf_dt, kind="ExternalInput").ap()

    task_d = nc.dram_tensor(
        "task_out", [UPC, MT, 128, D], f32, kind="ExternalOutput"
    ).ap()
    mean_d = nc.dram_tensor("mean_out", [MT, 128, D], f32, kind="ExternalOutput").ap()

    with tile.TileContext(nc) as tc:
        with (
            tc.tile_pool(name="const", bufs=1) as cpool,
            tc.tile_pool(name="wstream", bufs=4) as wpool,
            tc.tile_pool(name="out", bufs=4) as opool,
            tc.tile_pool(name="meanbuf", bufs=2) as mpool,
            tc.tile_pool(name="psum", bufs=8, space=bass.MemorySpace.PSUM) as ppool,
        ):
            # Resident constants: x^T tiles and the (host-pre-broadcast) bias.
            xt_sb = []
            for i in range(nxt):
                t = cpool.tile([128, KT, B], bf16, tag=f"xt{i}", name=f"xt{i}_sb")
                nc.sync.dma_start(t[:], xt_d[i][:])
                xt_sb.append(t)
            bias_sb = cpool.tile([128, UPC, D], f32, tag="bias")
            nc.sync.dma_start(bias_sb[:], bias_d[:])

            # Main per-user GEMM: accumulate over all k into 4 psum tiles
            # (2 row-tiles x 2 col-tiles), W streamed HBM->SBUF per k-block.
            for _rep in range(reps):
                for u in range(UPC):
                    ps = [
                        [
                            ppool.tile([128, NW], f32, tag="ps", name="ps")
                            for _ in range(NT)
                        ]
                        for _ in range(MT)
                    ]
                    for kb in range(NB):
                        w_sb = []
                        for wi in range(nwt):
                            t = wpool.tile(
                                [128, KB, D], bf16, tag=f"w{wi}", name=f"w{wi}_sb"
                            )
                            nc.sync.dma_start(t[:], w_d[wi][u, kb])
                            w_sb.append(t)
                        for ki in range(KB):
                            k = kb * KB + ki
                            for m in range(MT):
                                for pi, (xi, wi) in enumerate(passes):
                                    lhsT = xt_sb[xi][:, k, m * 128 : (m + 1) * 128]
                                    for n in range(NT):
                                        nc.tensor.matmul(
                                            ps[m][n][:],
                                            lhsT,
                                            w_sb[wi][:, ki, n * NW : (n + 1) * NW],
                                            start=(k == 0 and pi == 0),
                                            stop=(
                                                k == KT - 1 and pi == len(passes) - 1
                                            ),
                                        )
                    # Drain psum -> sbuf with bias add, then DMA out.
                    for m in range(MT):
                        ot = opool.tile([128, D], f32, tag="ot")
                        for n in range(NT):
                            sl = slice(n * NW, (n + 1) * NW)
                            nc.vector.tensor_add(
                                ot[:, sl], ps[m][n][:], bias_sb[:, u, sl]
                            )
                        nc.sync.dma_start(task_d[u, m], ot[:])

                # mean_emb = weight.mean(axis=1), fp32 end to end.
                for m in range(MT):
                    wf_sb = mpool.tile([128, IN], wf_dt, tag="wf")
                    nc.sync.dma_start(wf_sb[:], wf_d[m])
                    t0 = mpool.tile([128, D], f32, tag="mt0")
                    t1 = mpool.tile([128, D], f32, tag="mt1")
                    mo = mpool.tile([128, D], f32, tag="mo")
                    nc.vector.tensor_add(t0[:], wf_sb[:, 0:D], wf_sb[:, D : 2 * D])
                    nc.vector.tensor_add(
                        t1[:], wf_sb[:, 2 * D : 3 * D], wf_sb[:, 3 * D :]
                    )
                    nc.vector.tensor_add(t0[:], t0[:], t1[:])
                    nc.scalar.mul(mo[:], t0[:], 0.25)
                    nc.sync.dma_start(mean_d[m], mo[:])

    nc.compile()
    return nc


def _split_hi_lo(a):
    hi = a.astype(BF16)
    lo = (a - hi.astype(np.float32)).astype(BF16)
    return hi, lo


def kernel(weight, W_spec, b_spec):
    mode = MODE
    if mode not in _NC_CACHE:
        _NC_CACHE[mode] = _build_nc(mode)
    nc = _NC_CACHE[mode]

    weight = np.asarray(weight, dtype=np.float32)
    W_spec = np.asarray(W_spec, dtype=np.float32)
    b_spec = np.asarray(b_spec, dtype=np.float32)

    x = weight.reshape(B, IN)
    # xt[p, k, b] = x[b, 128k + p]
    xt_f32 = np.ascontiguousarray(x.T).reshape(KT, 128, B).transpose(1, 0, 2)
    if mode == "bf16x3":
        xh, xl = _split_hi_lo(xt_f32)
        xts = [np.ascontiguousarray(xh), np.ascontiguousarray(xl)]
        Wh, Wl = _split_hi_lo(W_spec)
        Ws = [Wh, Wl]
    else:
        mm_np = np.float16 if mode == "fp16" else BF16
        xts = [np.ascontiguousarray(xt_f32.astype(mm_np))]
        Ws = [W_spec.astype(mm_np)]

    wf = np.ascontiguousarray(
        weight.reshape(MT, 128, IN).astype(
            np.float16 if mode == "fp16" else np.float32
        )
    )

    in_maps = []
    for c in range(NCORES):
        us = slice(c * UPC, (c + 1) * UPC)
        m = {}
        for i, xti in enumerate(xts):
            m[f"xt{i}"] = xti
        for i, Wi in enumerate(Ws):
            # w[u, kb, p, ki*D + d] = W[c*UPC+u, (KB*kb+ki)*128 + p, d]
            wc = (
                Wi[us]
                .reshape(UPC, NB, KB, 128, D)
                .transpose(0, 1, 3, 2, 4)
                .reshape(UPC, NB, 128, KB * D)
            )
            m[f"w{i}"] = np.ascontiguousarray(wc)
        m["bias"] = np.ascontiguousarray(
            np.broadcast_to(b_spec[us][None, :, :], (128, UPC, D))
        )
        m["wf"] = wf
        in_maps.append(m)

    res = run_bass_kernel_spmd(
        nc,
        in_maps,
        core_ids=list(range(NCORES)),
        trace=bool(int(os.environ.get("CC_KERNEL_TRACE", "0"))),
    )
    _LAST_PERF.clear()
    _LAST_PERF.update(
        exec_time_ns=res.exec_time_ns,
        mean_exec_time_ns=res.mean_exec_time_ns,
        max_exec_time_core_id=res.max_exec_time_core_id,
        trace=res.instructions_and_trace[1] if res.instructions_and_trace else None,
    )

    task = np.concatenate(
        [res.results[c]["task_out"].reshape(UPC * B, D) for c in range(NCORES)], axis=0
    )
    mean = res.results[0]["mean_out"].reshape(B, D)
    return task.astype(np.float32, copy=False), mean.astype(np.float32, copy=False)
